# revision 50
# baseline (speedup 1.0000x reference)
"""KAN layer (nn_KANLayer) on 8 Trainium2 NeuronCores — Bass kernel, v4.

Data-parallel over batch (1024 rows/core).  Per core the contraction
y[b,j] = sum_{i,k} basis_k(tanh(x)_bi) * C[j,i,k] is a PE matmul with a
per-i-tile choice of feature basis (the weights absorb the change of
basis, so each 128-input tile picks whatever is cheapest to produce):

  pos 0   tile 0  fp16 min-ramps  M_m = min(xc - t_m, 0), 13 features.
                  1-op features: shortest possible DMA->tanh->feature->PE
                  chain from a standing start.
  pos 1-5 tiles 1-5  fp16 hats  B~_k = min(|xc - c_k|, dt) - dt, 12
                  features (12 chunks instead of 13 = ~8% less PE time).
                  ACT computes |xc-c_k| (k0-5), DVE finishes + builds
                  k6-9 whole (P/R clamps + max), Pool preps k10,k11.
  pos 6   tile 7  fp8e4 hats at 16x feature scale, matmuls in DoubleRow
                  perf mode (two 128-row chunks per instruction, 0.5
                  cycles/row).  Runs mid-stream so its tiny PE time does
                  not squeeze the next tile's feature window.  One fp8
                  tile costs ~1.4e-2 rel err of the 2e-2 budget (a second
                  would breach it).
  pos 7   tile 6  fp16 min-ramps again, bank-major: 1-op features keep
                  the tail off the DVE critical path, and each PSUM bank
                  stops + copies + DMAs out as soon as it completes
                  ((1,1) split into two 256-col banks to shorten the
                  final drain).

Engines run up to three tiles ahead of the PE (4-deep feature buffers);
each engine ends its per-tile block with a dummy semaphore bump so the
+1 write-drain margins resolve inside the tile instead of chaining
tiles together.  Warmup matmuls bridge the PE p-state ramp; x arrives
in column halves; weights in m-aligned chunks sized/ordered to land
just-in-time given the ~650ns/issue HWDGE serialization and 900ns DMA
semaphore propagation.  The ramp tail tile's weights sit on the wbufB
slot freed earliest so their DMA is not gated on late PE progress.

Measured (TimelineSim cost model): 84747 ns vs 98020 baseline (-13.5%).
Hardware rel err: 1.475e-02 (gate 2e-2).
Known dead ends (tested): quartering the first tanh/feature/matmul
group fails on hardware (corruption) despite simulating clean; sem-less
tail DMAs fail walrus codegen; SWDGE tail DMAs are slower (descriptor
generation runs on the Pool engine); a second fp8 tile breaches the
accuracy gate (~2.06% projected).
"""
import contextlib

import ml_dtypes
import numpy as np

import concourse.bass as bass
import concourse.mybir as mybir
from concourse import bass_utils

F32 = mybir.dt.float32
FP16 = mybir.dt.float16
BF16 = mybir.dt.bfloat16
FP8E4 = mybir.dt.float8e4
FSC = 16.0                  # fp8 tile feature scale (weights carry 1/16)

B, I, J = 8192, 1024, 256
NCORES = 8
BLOC = B // NCORES          # 1024 batch rows per core
NM0 = 13                    # tile-0 min-ramp features m = 1..13
NMH = 12                    # hat features k = 0..11 on tiles 1..7
NIT = I // 128              # 8 i-tiles
EPS = 1e-8
H = 512                     # column half

# tile-0 engine split (v2): DVE m0..m7, ACT tanh + m8,m9, GPS m10..m12
DVE_MS = list(range(0, 8))
ACT_MS = [8, 9]
GPS_MS = [10, 11, 12]

# hat-tile split: on fp16 tiles ACT computes |xc-c| for k0..5 and DVE
# self-computes k6..9 (cheaper in aggregate; ACT is the scarce engine);
# on the fp8 tile ACT has slack and takes k0..7.  Pool preps k10,k11.
ACT_KS16 = list(range(0, 6))
ACT_KS8 = list(range(0, 8))
ACT_KS = ACT_KS8  # superset, for bias consts / asc buffers
POOL_KS = [10, 11]

TUNE = {"nwarm": 4, "wwide": 512, "dma_order": "A", "fp8_t7": True}

_cached = None


def _knots64():
    return np.linspace(-1.0, 1.0, 16).astype(np.float32).astype(np.float64)


def _dt():
    return float(np.float32(2.0 / 15.0))


def _w_chunks():
    """Per-tile weight-column chunks (col unit = 128 = one (m,jh) block).
    Tile 0 fine-grained by first use (13 m = 26 units); fp16 hat tiles: 2
    chunks over 24 units; fp8 tile 7: one whole-buffer DMA."""
    t0 = [(0, 2), (2, 6), (6, 14), (14, 26)]
    rest = [(0, 12), (12, 24)]
    return ([t0] + [rest] * (NIT - 3) + [[(0, 14), (14, 26)]]
            + [[(0, 24)]])


def _zigzag0():
    """tile-0 (m, bh) group order: bh=1 groups trail by four slots."""
    order = [(m, 0) for m in range(4)]
    for m in range(4, NM0):
        order += [(m - 4, 1), (m, 0)]
    order += [(m, 1) for m in range(NM0 - 4, NM0)]
    return order


# tile-0 DVE emission order mirrors the PE need order for m0..m7
DVE_ORDER0 = ([(m, 0) for m in range(4)]
              + [x for m in range(4, 8) for x in ((m - 4, 1), (m, 0))]
              + [(m, 1) for m in range(4, 8)])

# hat-tile DVE feature completion orders (see vector thread)
DVE_IDX16 = {k: i + 1 for i, k in enumerate(
    [0, 1, 2, 3, 6, 4, 5, 7, 8, 9, 10, 11])}
DVE_IDX8 = {k: i + 1 for i, k in enumerate(
    [0, 1, 2, 3, 8, 4, 5, 9, 6, 7, 10, 11])}

# per-tile sem totals: tile 0, then per hat tile.  Each engine ends its
# hat-tile block with a tiny dummy increment so the +1 producer-op margins
# always resolve within the tile (no cross-tile serialization ladder).
# per-position sem increment counts.  positions: 0 = ramp tile 0,
# 1..5 = fp16 hat tiles, 6 = fp8 hat tile (engine-tile 7),
# 7 = fp16 RAMP tile (engine-tile 6, bank-major tail: 1-op features keep
# the tail off the DVE critical path).  Hat tiles append a dummy inc per
# engine so +1 margins resolve in-tile.
FV_CNT = [16] + [13] * 5 + [13] + [9]   # DVE completions (+dummy)
FA_CNT = [4] + [7] * 5 + [9] + [3]      # ACT abs/ramps (+dummy)
FG_CNT = [6] + [5] * 5 + [5] + [4]      # Pool (+dummy)
FA_CUM = [sum(FA_CNT[:p]) for p in range(len(FA_CNT) + 1)]
FV_CUM = [sum(FV_CNT[:p]) for p in range(len(FV_CNT) + 1)]
FG_CUM = [sum(FG_CNT[:p]) for p in range(len(FG_CNT) + 1)]
FV_TOT = FV_CUM[-1]
FA_TOT = FA_CUM[-1]
FG_TOT = FG_CUM[-1]


def _fv_base(pos):
    return FV_CUM[pos]


def _fa_base(pos):
    return FA_CUM[pos]


def _fg_base(pos):
    return FG_CUM[pos]


# stream order: the fp8 DoubleRow tile (engine-tile 7) runs at position 6
# so the fp16 tile 6 is last (its bank-major tail overlaps copies); engines
# produce in the same order so the pipeline stays aligned
ORDER = [0, 1, 2, 3, 4, 5, 7, 6]


def _xc_tot(i):
    """total s_xc increments through tile i (tile0: 2 halves, rest: 1 full)"""
    return 2 + i


def _build():
    kn = _knots64()
    dt = _dt()
    thr = [float(np.float32(kn[m])) for m in range(1, 15)]  # ramp thresholds
    ctr = [float(np.float32(kn[k + 1])) for k in range(NMH)]  # hat centers

    nc = bass.Bass("TRN2", target_bir_lowering=False, debug=False,
                   num_devices=NCORES)

    xd = nc.declare_dram_parameter("x", [I, BLOC], F32, isOutput=False)
    w0d = nc.declare_dram_parameter("w0", [128, NM0 * J], FP16, isOutput=False)
    wd = nc.declare_dram_parameter("w", [NIT - 3, 128, NMH * J], FP16,
                                   isOutput=False)
    w6d = nc.declare_dram_parameter("w6", [128, NM0 * J], FP16, isOutput=False)
    w8d = nc.declare_dram_parameter("w8", [128, NMH // 2, 2, J], FP8E4,
                                    isOutput=False)
    yd = nc.declare_dram_parameter("y", [J, BLOC], F32, isOutput=True)

    # const APs for ACT bias values; memsets run on the Pool queue and are
    # sem-gated (no global barrier).
    b16 = [float(np.float32(-FSC) * np.float32(ctr[k])) for k in range(NMH)]
    cvals = ([thr[m] for m in ACT_MS] + [-ctr[k] for k in ACT_KS]
             + [b16[k] for k in ACT_KS])
    cbuf = []
    for ci, v in enumerate(cvals):
        t = nc.alloc_sbuf_tensor(f"const-{ci}", [128, 1], F32)
        nc.const_aps.aps[(F32, v)] = t.ap()
        cbuf.append((t, v))
    ninit = len(cvals) + 1  # + wz memset

    ctx = contextlib.ExitStack()
    xbuf = [ctx.enter_context(nc.sbuf_tensor(f"xbuf{p}", [128, BLOC], F32))
            for p in range(2)]
    xcb = [ctx.enter_context(nc.sbuf_tensor(f"xcb{p}", [128, BLOC], FP16))
           for p in range(2)]
    # 4-deep: engines run up to three tiles ahead of the PE (needed to
    # absorb the fp8 tile's feature burst)
    fbufB = [[ctx.enter_context(nc.sbuf_tensor(f"fbufB{k}_{q}", [128, BLOC], FP16))
              for q in range(4)] for k in range(NMH)]
    # tile-0 ramp features alias the q=3 slots: those are first rewritten by
    # DVE at stream pos 4, which is gated on s_pe >= 1 (PE done with tile 0)
    fbuf0 = [fbufB[m][3] for m in range(NMH)] + [ctx.enter_context(
        nc.sbuf_tensor("fbuf0_12", [128, BLOC], FP16))]
    # k6,k7 abs are only produced on the fp8 tile -> no parity needed
    ascb = [[ctx.enter_context(nc.sbuf_tensor(f"asc{k}_{p}", [128, BLOC], FP16))
             for p in range(2)] if k < 6 else
            [ctx.enter_context(nc.sbuf_tensor(f"asc{k}", [128, BLOC], FP16))] * 2
            for k in ACT_KS]
    # nxv/pxg hold 16*xc on the fp8 tile only -> no parity needed
    nxv = [ctx.enter_context(nc.sbuf_tensor("nxv", [128, BLOC], FP16))] * 2
    rxv = [ctx.enter_context(nc.sbuf_tensor(f"rxv{p}", [128, BLOC], FP16))
           for p in range(2)]
    nxg = [ctx.enter_context(nc.sbuf_tensor(f"nxg{p}", [128, BLOC], FP16))
           for p in range(2)]
    pxg = [ctx.enter_context(nc.sbuf_tensor("pxg", [128, BLOC], FP16))] * 2
    psc = ctx.enter_context(nc.sbuf_tensor("psc", [128, BLOC], FP16))
    rsc = ctx.enter_context(nc.sbuf_tensor("rsc", [128, BLOC], FP16))
    dum = {e: ctx.enter_context(nc.sbuf_tensor(f"dum{e}", [128, 8], FP16))
           for e in ("v", "a", "g")}
    prg = {(k, w): [ctx.enter_context(
        nc.sbuf_tensor(f"prg{k}{w}_{p}", [128, BLOC], FP16)) for p in range(2)]
        for k in POOL_KS for w in ("p", "r")}
    fpair = ctx.enter_context(
        nc.sbuf_tensor("fpair", [128, NMH // 2, 2, BLOC], FP8E4))
    wbuf8 = ctx.enter_context(
        nc.sbuf_tensor("wbuf8", [128, NMH // 2, 2, J], FP8E4))
    wbuf0 = ctx.enter_context(nc.sbuf_tensor("wbuf0", [128, NM0 * J], FP16))
    # wbufB[1] is sized for the 13-m ramp tail tile (stream pos 7), whose
    # previous user is stream pos 4 -> its DMA only waits s_pe >= 5
    wbufB = [ctx.enter_context(nc.sbuf_tensor(
        f"wbufB{q}", [128, (NMH if q == 0 else NM0) * J], FP16))
        for q in range(2)]
    nwarm = TUNE["nwarm"]
    wwide = TUNE["wwide"]
    wz = (ctx.enter_context(nc.sbuf_tensor("wz", [128, wwide], BF16))
          if nwarm else None)
    ps = [[ctx.enter_context(nc.psum_tensor(f"ps{jh}_{bh}", [128, 512], F32))
           for bh in range(2)] for jh in range(2)]
    # (jh=1, bh=1) accumulates as two 256-col banks: independent stop+drain
    # at the tail.  matmuls write and copies read WHOLE tensors (partial
    # PSUM reads crash the device runtime).
    ps11 = [ctx.enter_context(nc.psum_tensor(f"ps11{r}", [128, 256], F32))
            for r in range(2)]
    psw = (ctx.enter_context(nc.psum_tensor("psw", [128, wwide], F32))
           if nwarm else None)
    # output staging reuses xbuf: x is fully consumed long before the tail
    obuf = xbuf

    chunks = _w_chunks()
    # global chunk idx gating (i, m, jh); chunks are issued in stream ORDER
    wneed = {}
    g = 0
    for pos in range(NIT):
        i = ORDER[pos]
        for (c0, c1) in chunks[i]:
            for col in range(c0, c1):
                wneed[(i, col // 2, col % 2)] = g
            g += 1
    nchunks = g
    zz0 = _zigzag0()

    npx = 2 * NIT  # x DMA half-transfers, indexed by stream position

    with ctx:
        xsems = [ctx.enter_context(nc.semaphore(name=f"s_xd{k}"))
                 for k in range(npx)]
        wsems = [ctx.enter_context(nc.semaphore(name=f"s_wd{c}"))
                 for c in range(nchunks)]
        with (
            nc.semaphore() as s_init,
            nc.semaphore() as s_ydone,
            nc.semaphore() as s_xc,
            nc.semaphore() as s_fv,
            nc.semaphore() as s_fa,
            nc.semaphore() as s_fg,
            nc.semaphore() as s_pe,
            nc.semaphore() as s_bank,
            nc.semaphore() as s_cpv,
            nc.semaphore() as s_cpa,
            nc.Block() as block,
        ):
            @block.sync
            def _(sync):
                def xdma(i, p, b0, b1, part):
                    sync.dma_start(
                        out=xbuf[p][:, b0:b1],
                        in_=xd[i * 128:(i + 1) * 128, b0:b1],
                    ).then_inc(xsems[part], 16)

                wcnt = [0]

                def wdma(i, c0, c1):
                    c = wcnt[0]
                    wcnt[0] += 1
                    if i == 0:
                        sync.dma_start(
                            out=wbuf0[:, c0 * 128:c1 * 128],
                            in_=w0d[:, c0 * 128:c1 * 128],
                        ).then_inc(wsems[c], 16)
                    elif i == NIT - 1:
                        sync.dma_start(out=wbuf8[:], in_=w8d[:],
                                       ).then_inc(wsems[c], 16)
                    elif i == NIT - 2:
                        sync.dma_start(
                            out=wbufB[1][:, c0 * 128:c1 * 128],
                            in_=w6d[:, c0 * 128:c1 * 128],
                        ).then_inc(wsems[c], 16)
                    else:
                        sync.dma_start(
                            out=wbufB[(i - 1) % 2][:, c0 * 128:c1 * 128],
                            in_=wd[i - 1][:, c0 * 128:c1 * 128],
                        ).then_inc(wsems[c], 16)

                # tile 0: x halves interleaved with w chunks so each lands
                # just-in-time given the ~650ns/issue HWDGE serialization
                t0ops = {
                    "xh0": lambda: xdma(0, 0, 0, 512, 0),
                    "xh1": lambda: xdma(0, 0, 512, 1024, 1),
                    "w0": lambda: wdma(0, *chunks[0][0]),
                    "w1": lambda: wdma(0, *chunks[0][1]),
                    "w2": lambda: wdma(0, *chunks[0][2]),
                    "w3": lambda: wdma(0, *chunks[0][3]),
                }
                orders = {
                    "A": ["xh0", "w0", "xh1", "w1", "w2", "w3"],
                    "B": ["xh0", "xh1", "w0", "w1", "w2", "w3"],
                    "C": ["xh0", "w0", "w1", "xh1", "w2", "w3"],
                }
                for name in orders[TUNE["dma_order"]]:
                    t0ops[name]()
                for pos in range(1, NIT):
                    i = ORDER[pos]
                    p = pos % 2
                    if pos >= 2:
                        sync.wait_ge(s_xc, _xc_tot(pos - 2))  # xbuf[p] free
                    xdma(i, p, 0, 512, 2 * pos)
                    xdma(i, p, 512, 1024, 2 * pos + 1)
                    if pos >= 3 and i != NIT - 1:
                        # wbufB free: pos-7 (ramp tile) sits on wbufB[1],
                        # whose previous user is pos 4
                        sync.wait_ge(s_pe, 5 if i == NIT - 2 else pos - 1)
                    for (c0, c1) in chunks[i]:
                        wdma(i, c0, c1)
                # output DMAs, one per PSUM bank, in bank completion order
                for (waits, jh, c0, c1) in (
                        (((s_cpv, 1),), 0, 0, 512),
                        (((s_cpa, 1),), 1, 0, 512),
                        (((s_cpv, 2),), 0, 512, 1024),
                        (((s_cpa, 2),), 1, 512, 768),
                        (((s_cpv, 3),), 1, 768, 1024)):
                    for (sem, val) in waits:
                        sync.wait_ge(sem, val)
                    sync.dma_start(
                        out=yd[jh * 128:(jh + 1) * 128, c0:c1],
                        in_=obuf[jh][:, c0:c1],
                    ).then_inc(s_ydone, 16)

            @block.scalar
            def _(scalar):
                # ---- tile 0: tanh halves + ramps m8,m9 ----
                for (part, b0, b1) in ((0, 0, 512), (1, 512, 1024)):
                    scalar.wait_ge(xsems[part], 16)
                    nc.scalar.activation(
                        xcb[0][:, b0:b1], xbuf[0][:, b0:b1],
                        mybir.ActivationFunctionType.Tanh,
                    ).then_inc(s_xc, 1)
                scalar.wait_ge(s_init, ninit)
                for m in ACT_MS:
                    for h in range(2):
                        nc.scalar.activation(
                            fbuf0[m][:, h * H:(h + 1) * H],
                            xcb[0][:, h * H:(h + 1) * H],
                            mybir.ActivationFunctionType.Relu,
                            bias=thr[m], scale=-1.0,
                        ).then_inc(s_fa, 1)
                # ---- hat tiles: full-width tanh + abs k0..6 ----
                for pos in range(1, NIT):
                    i = ORDER[pos]
                    p = pos % 2
                    scalar.wait_ge(xsems[2 * pos], 16)
                    scalar.wait_ge(xsems[2 * pos + 1], 16)
                    if pos >= 2:
                        # xcb[p]/asc[*][p] free: tile i-2 consumed them.
                        # DVE's last xcb read is self_hat(9)'s P op (10th
                        # completion); Pool's is P11 (3rd) — excluding the
                        # trailing tt/dummy ops avoids chaining tiles.
                        scalar.wait_ge(s_fv, 16 if pos == 2
                                       else _fv_base(pos - 1) - 3)
                        scalar.wait_ge(s_fg, 6 if pos == 2
                                       else _fg_base(pos - 1) - 2)
                    nc.scalar.activation(
                        xcb[p][:], xbuf[p][:],
                        mybir.ActivationFunctionType.Tanh,
                    ).then_inc(s_xc, 1)
                    t7 = (i == NIT - 1)
                    if i == NIT - 2:
                        # ramp tail tile: relu(t_m - xc) for m8,m9
                        for m in ACT_MS:
                            nc.scalar.activation(
                                fbufB[m][(pos - 1) % 4][:], xcb[p][:],
                                mybir.ActivationFunctionType.Relu,
                                bias=thr[m], scale=-1.0,
                            ).then_inc(s_fa, 1)
                    else:
                        for k in (ACT_KS8 if t7 else ACT_KS16):
                            nc.scalar.activation(
                                ascb[k][p][:], xcb[p][:],
                                mybir.ActivationFunctionType.Abs,
                                bias=(b16[k] if t7 else -ctr[k]),
                                scale=(FSC if t7 else 1.0),
                            ).then_inc(s_fa, 1)
                    nc.scalar.activation(
                        dum["a"][:], dum["a"][:],
                        mybir.ActivationFunctionType.Copy,
                    ).then_inc(s_fa, 1)
                # ---- output copies: (j1,b0) and (j1,b1a) ----
                scalar.wait_ge(s_bank, 2)
                nc.scalar.copy(obuf[1][:, 0:512], ps[1][0][:]).then_inc(s_cpa, 1)
                scalar.wait_ge(s_bank, 4)
                nc.scalar.copy(obuf[1][:, 512:768],
                               ps11[0][:]).then_inc(s_cpa, 1)

            @block.vector
            def _(vector):
                TS = nc.vector.tensor_scalar
                # ---- tile 0: min-ramp features m0..m7 per half ----
                waited = [False, False]
                for (m, h) in DVE_ORDER0:
                    if not waited[h]:
                        vector.wait_ge(s_xc, h + 1)
                        waited[h] = True
                    TS(fbuf0[m][:, h * H:(h + 1) * H],
                       xcb[0][:, h * H:(h + 1) * H], thr[m], 0.0,
                       mybir.AluOpType.subtract, mybir.AluOpType.min,
                       ).then_inc(s_fv, 1)
                # ---- hat tiles ----
                for pos in range(1, NIT):
                    i = ORDER[pos]
                    p = pos % 2
                    q = (pos - 1) % 4
                    t7 = (i == NIT - 1)
                    sc = FSC if t7 else 1.0
                    if pos >= 4 and not t7:
                        vector.wait_ge(s_pe, pos - 3)  # fbufB[q] free
                    vector.wait_ge(s_xc, _xc_tot(pos))

                    def fout(k):
                        # feature destination: fp8 pair buffer on tile 7
                        if t7:
                            return fpair[:, k // 2, k % 2, :]
                        return fbufB[k][q][:]

                    def fin(k):
                        vector.wait_ge(s_fa, min(
                            _fa_base(pos) + (k + 1) + 1, FA_TOT))
                        TS(fout(k), ascb[k][p][:], sc * dt, sc * dt,
                           mybir.AluOpType.min, mybir.AluOpType.subtract,
                           ).then_inc(s_fv, 1)

                    def self_hat(k):
                        TS(psc[:], (nxv[p] if t7 else xcb[p])[:],
                           sc * (ctr[k] + dt), sc * (ctr[k] + dt),
                           mybir.AluOpType.min, mybir.AluOpType.subtract)
                        TS(rsc[:], rxv[p][:], sc * (dt - ctr[k]),
                           sc * (dt - ctr[k]),
                           mybir.AluOpType.min, mybir.AluOpType.subtract)
                        nc.vector.tensor_tensor(
                            fout(k), psc[:], rsc[:],
                            mybir.AluOpType.max).then_inc(s_fv, 1)

                    if i == NIT - 2:
                        # ramp tail tile: min(xc - t_m, 0) for m0..7
                        for m in DVE_MS:
                            TS(fbufB[m][q][:], xcb[p][:], thr[m], 0.0,
                               mybir.AluOpType.subtract, mybir.AluOpType.min,
                               ).then_inc(s_fv, 1)
                        TS(dum["v"][:], dum["v"][:], 1.0, 0.0,
                           mybir.AluOpType.mult, mybir.AluOpType.add,
                           ).then_inc(s_fv, 1)
                        continue
                    # nxv = sc*xc (only needed scaled on tile 7; on fp16
                    # tiles P reads xcb directly), rxv = -sc*xc
                    if t7:
                        TS(nxv[p][:], xcb[p][:], FSC, 0.0,
                           mybir.AluOpType.mult, mybir.AluOpType.add)
                    TS(rxv[p][:], xcb[p][:], -sc, 0.0,
                       mybir.AluOpType.mult, mybir.AluOpType.add)
                    fin(0), fin(1), fin(2), fin(3)
                    if t7:
                        self_hat(8)
                        fin(4), fin(5)
                        self_hat(9)
                        fin(6), fin(7)
                    else:
                        self_hat(6)
                        fin(4), fin(5)
                        self_hat(7), self_hat(8), self_hat(9)
                    for k in POOL_KS:
                        vector.wait_ge(s_fg, min(
                            _fg_base(pos) + 2 * (k - 9) + 1, FG_TOT))
                        nc.vector.tensor_tensor(
                            fout(k), prg[(k, "p")][p][:],
                            prg[(k, "r")][p][:],
                            mybir.AluOpType.max).then_inc(s_fv, 1)
                    TS(dum["v"][:], dum["v"][:], 1.0, 0.0,
                       mybir.AluOpType.mult, mybir.AluOpType.add,
                       ).then_inc(s_fv, 1)
                # ---- output copies: (j0,b0), (j0,b1), (j1,b1b) ----
                vector.wait_ge(s_bank, 1)
                nc.vector.tensor_copy(obuf[0][:, 0:512],
                                      ps[0][0][:]).then_inc(s_cpv, 1)
                vector.wait_ge(s_bank, 3)
                nc.vector.tensor_copy(obuf[0][:, 512:1024],
                                      ps[0][1][:]).then_inc(s_cpv, 1)
                vector.wait_ge(s_bank, 5)
                nc.vector.tensor_copy(obuf[1][:, 768:1024],
                                      ps11[1][:]).then_inc(s_cpv, 1)

            @block.gpsimd
            def _(gpsimd):
                TS = nc.gpsimd.tensor_scalar
                if nwarm:
                    nc.gpsimd.memset(wz[:], 0.0).then_inc(s_init, 1)
                for (t, v) in cbuf:
                    nc.gpsimd.memset(t.ap(), v).then_inc(s_init, 1)
                # ---- tile 0: min-ramp features m10..m12 per half ----
                for h in range(2):
                    gpsimd.wait_ge(s_xc, h + 1)
                    for m in GPS_MS:
                        TS(fbuf0[m][:, h * H:(h + 1) * H],
                           xcb[0][:, h * H:(h + 1) * H], thr[m], 0.0,
                           mybir.AluOpType.subtract, mybir.AluOpType.min,
                           ).then_inc(s_fg, 1)
                # ---- hat tiles: own -xc, then P/R for k=10,11 ----
                for pos in range(1, NIT):
                    i = ORDER[pos]
                    p = pos % 2
                    t7 = (i == NIT - 1)
                    sc = FSC if t7 else 1.0
                    if pos >= 2:
                        # prg free: DVE pos-2's tt11 is its 12th completion
                        gpsimd.wait_ge(s_fv, 16 if pos == 2
                                       else _fv_base(pos - 1) - 1)
                    gpsimd.wait_ge(s_xc, _xc_tot(pos))
                    if i == NIT - 2:
                        # ramp tail tile: min(xc - t_m, 0) for m10..12
                        for m in GPS_MS:
                            dst = (fbufB[m][(pos - 1) % 4] if m < NMH
                                   else fbuf0[12])
                            TS(dst[:], xcb[p][:], thr[m], 0.0,
                               mybir.AluOpType.subtract, mybir.AluOpType.min,
                               ).then_inc(s_fg, 1)
                        TS(dum["g"][:], dum["g"][:], 1.0, 0.0,
                           mybir.AluOpType.mult, mybir.AluOpType.add,
                           ).then_inc(s_fg, 1)
                        continue
                    if t7:
                        TS(pxg[p][:], xcb[p][:], FSC, 0.0,
                           mybir.AluOpType.mult, mybir.AluOpType.add)
                    TS(nxg[p][:], xcb[p][:], -sc, 0.0,
                       mybir.AluOpType.mult, mybir.AluOpType.add)
                    for k in POOL_KS:
                        TS(prg[(k, "p")][p][:], (pxg[p] if t7 else xcb[p])[:],
                           sc * (ctr[k] + dt), sc * (ctr[k] + dt),
                           mybir.AluOpType.min, mybir.AluOpType.subtract,
                           ).then_inc(s_fg, 1)
                        TS(prg[(k, "r")][p][:], nxg[p][:],
                           sc * (dt - ctr[k]), sc * (dt - ctr[k]),
                           mybir.AluOpType.min, mybir.AluOpType.subtract,
                           ).then_inc(s_fg, 1)
                    TS(dum["g"][:], dum["g"][:], 1.0, 0.0,
                       mybir.AluOpType.mult, mybir.AluOpType.add,
                       ).then_inc(s_fg, 1)

            @block.tensor
            def _(tensor):
                if nwarm:
                    tensor.wait_ge(s_init, 1)
                for k in range(nwarm):
                    nc.tensor.matmul(psw[:], wz[:, :128], wz[:],
                                     start=True, stop=True)

                def feat_val(pos, m, h):
                    """(sem, value) gating feature m (half h; hat tiles are
                    full-width so h is ignored past tile 0).  +1 producer-op
                    margin: the producing engine is in-order, so the next
                    op's sem guarantees this op's SBUF write has drained."""
                    if pos == 0:
                        if m in DVE_MS:
                            idx = DVE_ORDER0.index((m, h)) + 1
                            return (s_fv, min(idx + 1, FV_TOT))
                        if m in ACT_MS:
                            v = 2 * (m - 8) + h + 1
                            return (s_fa, min(v + 1, FA_TOT))
                        v = 3 * h + (m - 10) + 1
                        return (s_fg, min(v + 1, FG_TOT))
                    idx = DVE_IDX8 if pos == NIT - 2 else DVE_IDX16
                    v = _fv_base(pos) + idx[m]
                    return (s_fv, min(v + 1, FV_TOT))

                def emit(pos, m, bh, jh, c0, c1, start, stop, wait=None):
                    if wait is not None:
                        tensor.wait_ge(wait[0], wait[1])
                    if (jh, bh) == (1, 1):
                        out = ps11[0 if c0 == 0 else 1][:]
                    else:
                        out = ps[jh][bh][:, c0:c1]
                    if pos == 0:
                        lhsT = wbuf0[:, (m * 2 + jh) * 128:(m * 2 + jh + 1) * 128]
                        rhs = fbuf0[m][:, bh * H + c0:bh * H + c1]
                    else:
                        lhsT = wbufB[(pos - 1) % 2][:, (m * 2 + jh) * 128:
                                                    (m * 2 + jh + 1) * 128]
                        rhs = fbufB[m][(pos - 1) % 4][:, bh * H + c0:bh * H + c1]
                    return nc.tensor.matmul(out, lhsT, rhs,
                                            start=start, stop=stop)

                # ---- tile 0: zig-zag (m, bh) groups ----
                # chunk gates: chunk idx first needed at m (jh=1 covers both)
                gate0 = {}
                seen = -1
                for m in range(NM0):
                    v = wneed[(0, m, 1)]
                    if v > seen:
                        gate0[m] = v
                        seen = v
                tensor.wait_ge(wsems[wneed[(0, 0, 0)]], 16)
                for gi, (m, bh) in enumerate(zz0):
                    if bh == 0 and m in gate0 and m != 0:
                        tensor.wait_ge(wsems[gate0[m]], 16)
                    start = (m == 0)
                    emit(0, m, bh, 0, 0, 512, start, False,
                         feat_val(0, m, bh))
                    if bh == 0:
                        ins = emit(0, m, bh, 1, 0, 512, start, False)
                    else:
                        emit(0, m, bh, 1, 0, 256, start, False)
                        ins = emit(0, m, bh, 1, 256, 512, start, False)
                ins.then_inc(s_pe, 1)

                # ---- hat tiles, stream positions 1..5: m-major ----
                for pos in range(1, NIT - 2):
                    base_chunk = 4 + 2 * (pos - 1)
                    for m in range(NMH):
                        if m == 0:
                            tensor.wait_ge(wsems[base_chunk], 16)
                        elif m == 6:
                            tensor.wait_ge(wsems[base_chunk + 1], 16)
                        emit(pos, m, 0, 0, 0, 512, False, False,
                             feat_val(pos, m, 0))
                        emit(pos, m, 0, 1, 0, 512, False, False)
                        emit(pos, m, 1, 0, 0, 512, False, False)
                        emit(pos, m, 1, 1, 0, 256, False, False)
                        ins = emit(pos, m, 1, 1, 256, 512, False, False)
                    ins.then_inc(s_pe, 1)

                # ---- fp8 DoubleRow tile (engine-tile 7), stream pos 6:
                # accumulation order is free, and placing the fast tile here
                # keeps the slower fp16 tile 6 last so the tail's bank
                # copies overlap its matmuls ----
                pos = NIT - 2
                tensor.wait_ge(wsems[14], 16)
                for pr in range(NMH // 2):
                    v = _fv_base(pos) + max(DVE_IDX8[2 * pr],
                                            DVE_IDX8[2 * pr + 1])
                    tensor.wait_ge(s_fv, min(v + 1, FV_TOT))
                    for (bh, jh, c0, c1) in ((0, 0, 0, 512), (0, 1, 0, 512),
                                             (1, 0, 0, 512), (1, 1, 0, 256),
                                             (1, 1, 256, 512)):
                        if (jh, bh) == (1, 1):
                            out = ps11[0 if c0 == 0 else 1][:]
                        else:
                            out = ps[jh][bh][:, c0:c1]
                        ins = nc.tensor.matmul(
                            out,
                            wbuf8[:, pr, :, jh * 128:(jh + 1) * 128],
                            fpair[:, pr, :, bh * H + c0:bh * H + c1],
                            start=False, stop=False,
                            perf_mode=mybir.MatmulPerfMode.DoubleRow)
                ins.then_inc(s_pe, 1)

                # ---- last tile (engine-tile 6), stream pos 7: 13-m RAMP
                # tile, bank-major so banks finish (and drain) early.  The
                # 1-op ramp features keep the tail off the DVE critical path.
                pos = NIT - 1
                tensor.wait_ge(wsems[nchunks - 1], 16)
                banks = [(0, 0, 0, 512), (0, 1, 0, 512), (1, 0, 0, 512),
                         (1, 1, 0, 256), (1, 1, 256, 512)]

                def rampw(m, h):
                    if m in DVE_MS:
                        return (s_fv, min(_fv_base(pos) + m + 2, FV_TOT))
                    if m in ACT_MS:
                        return (s_fa, min(_fa_base(pos) + (m - 8) + 2, FA_TOT))
                    return (s_fg, min(_fg_base(pos) + (m - 10) + 2, FG_TOT))

                for bi, (bh, jh, c0, c1) in enumerate(banks):
                    for m in range(NM0):
                        if bi == 0:
                            w8v = rampw(m, bh)
                            tensor.wait_ge(w8v[0], w8v[1])
                        if (jh, bh) == (1, 1):
                            out = ps11[0 if c0 == 0 else 1][:]
                        else:
                            out = ps[jh][bh][:, c0:c1]
                        rhs = (fbufB[m][(pos - 1) % 4] if m < NMH
                               else fbuf0[12])
                        ins = nc.tensor.matmul(
                            out,
                            wbufB[1][:, (m * 2 + jh) * 128:
                                     (m * 2 + jh + 1) * 128],
                            rhs[:, bh * H + c0:bh * H + c1],
                            start=False, stop=(m == NM0 - 1))
                    ins.then_inc(s_bank, 1)

    return nc


def _ramp_weights(C, kn, i0):
    """v2 min-ramp weights (second differences, sign per engine) for the
    128 inputs starting at i0."""
    s = np.array([0.5 * (1.0 / (kn[k + 1] - kn[k] + EPS)
                         + 1.0 / (kn[k + 2] - kn[k + 1] + EPS))
                  for k in range(12)])
    Cp = C[:, i0:i0 + 128, :12] * s[None, None, :]
    W = np.zeros((128, NM0, J))                        # index 0 -> m=1
    for mi in range(NM0):
        m = mi + 1
        acc = np.zeros((J, 128))
        if m <= 11:
            acc += Cp[:, :, m]
        if 0 <= m - 1 <= 11:
            acc -= 2.0 * Cp[:, :, m - 1]
        if 0 <= m - 2 <= 11:
            acc += Cp[:, :, m - 2]
        W[:, mi, :] = acc.T if mi in ACT_MS else -acc.T
    return W.reshape(128, NM0, 2, 128).reshape(128, NM0 * J)


def _weights(spline_coeffs, knots=None):
    """Tiles 0 and 6: min-ramp weights.  Tiles 1-5: fp16 hat weights -C/dt.
    Tile 7: fp8 hat weights -C/dt/16 (features carry the 16x)."""
    kn = _knots64()
    dt = _dt()
    C = np.asarray(spline_coeffs, np.float64)          # [J, I, 13]
    W0 = _ramp_weights(C, kn, 0)
    W6 = _ramp_weights(C, kn, 768)

    Wh = -C[:, 128:768, :NMH] / dt                     # [J, 640, 12]
    Wh = np.transpose(Wh, (1, 2, 0))                   # [640, 12, J]
    Wh = Wh.reshape(NIT - 3, 128, NMH, 2, 128).reshape(NIT - 3, 128, NMH * J)
    W8 = -C[:, 896:, :NMH] / dt / FSC                  # [J, 128, 12]
    W8 = np.transpose(W8, (1, 2, 0))                   # [128, 12, J]
    W8 = np.clip(W8, -240.0, 240.0).reshape(128, NMH // 2, 2, J)
    return (np.ascontiguousarray(W0, dtype=np.float16),
            np.ascontiguousarray(Wh, dtype=np.float16),
            np.ascontiguousarray(W6, dtype=np.float16),
            np.ascontiguousarray(W8.astype(ml_dtypes.float8_e4m3)))


def _in_maps(x, spline_coeffs, knots=None):
    W0, Wh, W6, W8 = _weights(spline_coeffs, knots)
    in_maps = []
    for c in range(NCORES):
        xT = np.ascontiguousarray(x[c * BLOC:(c + 1) * BLOC, :].T)  # [I, BLOC]
        in_maps.append({"x": xT, "w0": W0, "w": Wh, "w6": W6, "w8": W8})
    return in_maps


def kernel(x, spline_coeffs, knots):
    global _cached
    x = np.asarray(x, dtype=np.float32)
    spline_coeffs = np.asarray(spline_coeffs, dtype=np.float32)

    if _cached is None:
        _cached = _build()
    nc = _cached

    in_maps = _in_maps(x, spline_coeffs, knots)

    res = bass_utils.run_bass_kernel_spmd(nc, in_maps,
                                          core_ids=list(range(NCORES)))
    out = np.empty((B, J), dtype=np.float32)
    for c in range(NCORES):
        out[c * BLOC:(c + 1) * BLOC, :] = res.results[c]["y"].T
    return out
